# revision 17
# baseline (speedup 1.0000x reference)
"""MoE (8 routed experts, top-2, + shared expert) on 8 TRN2 NeuronCores.

Strategy: expert-parallel. Host computes the gate (fp32 numpy, exactly
mirroring the reference), gathers each expert's tokens, and core e runs
expert e's SwiGLU (h = silu(x@w1T) * (x@w3T) * cw; y = h_bf16 @ w2T)
over its gathered tokens, plus a 1/8 token-slice of the shared expert.
Host scatters expert outputs back and combines in bf16 expert order.

All tensors fed to the device are pre-arranged on host into
partition-major layouts so every DMA is contiguous per partition:
  activations/weights for matmul lhsT/rhs always have the contraction
  dim chunked as [pi=128, po, free].
"""

import numpy as np
import ml_dtypes

import concourse.mybir as mybir
from concourse import bacc
from concourse.tile import TileContext
from concourse import bass_utils

BF16 = mybir.dt.bfloat16
F32 = mybir.dt.float32

D = 2048          # model dim
I = 1408          # expert inter dim
E = 8             # routed experts
TOPK = 2
N_CORES = 8
DPO = D // 128    # 16 chunks of the model dim
IPO = I // 128    # 11 chunks of the inter dim

_BUILD_CACHE = {}


def _c_blocks(C):
    """Split C columns into equal-ish blocks <= 512, multiples of 128."""
    nb = -(-C // 512)
    per = -(-C // (nb * 128)) * 128
    blocks = []
    off = 0
    while off < C:
        w = min(per, C - off)
        blocks.append((off, w))
        off += w
    return blocks


def _build(C, TS):
    """Build the per-core Bass kernel for routed capacity C and shared
    token-slice TS. Same NEFF runs SPMD on all 8 cores."""
    nc = bacc.Bacc("TRN2", debug=False, enable_asserts=False,
                   num_devices=N_CORES, enable_partition_id=False)

    def din(name, shape, dt=BF16):
        return nc.dram_tensor(name, shape, dt, kind="ExternalInput").ap()

    def dout(name, shape, dt=BF16):
        return nc.dram_tensor(name, shape, dt, kind="ExternalOutput").ap()

    xr = din("xr", [128, DPO, C])            # routed tokens, [d_pi, d_po, c]
    xs = din("xs", [128, DPO, TS])           # shared-expert token slice
    cwb = din("cwb", [128, C], F32)          # combine weights, replicated
    w1t = din("w1t", [IPO, 128, D])          # [i_blk][d_pi][d_po*128+i_c]
    w3t = din("w3t", [IPO, 128, D])
    w2t = din("w2t", [DPO, 128, I])          # [d_blk][i_pi][i_po*128+d_c]
    sw1t = din("sw1t", [IPO, 128, D])
    sw3t = din("sw3t", [IPO, 128, D])
    sw2t = din("sw2t", [DPO, 128, I])
    ye = dout("ye", [128, DPO, C])           # [d_pi, d_po, c]
    zs = dout("zs", [128, DPO, TS])

    Silu = mybir.ActivationFunctionType.Silu

    with TileContext(nc) as tc:
        with tc.tile_pool(name="main", bufs=1) as pool, \
             tc.tile_pool(name="psum", bufs=1, space="PSUM") as pp:
            cw_sb = pool.tile([128, C], F32, tag="cwb", bufs=1, name="cw_sb")

            # shared job first: its startup needs ~1.1MB before the first
            # matmul (vs 1.3MB + a 4.7MB x stream for routed), so the
            # HBM-bound head shrinks; routed x streams during shared work
            jobs = [
                ("s", TS, xs, sw1t, sw3t, sw2t, zs, False),
                ("r", C, xr, w1t, w3t, w2t, ye, True),
            ]
            for jname, CJ, x_d, w1_d, w3_d, w2_d, out_d, scaled in jobs:
                cbs = _c_blocks(CJ)
                x_sb = pool.tile([128, DPO, CJ], BF16, tag=f"x_{jname}",
                                 bufs=1, name=f"x_{jname}")
                # startup: land just enough bytes for the first matmuls
                # (x slice 0 + the first weight chunks) before streaming
                # the rest, so the PE starts ~10us in instead of ~25us
                nc.sync.dma_start(x_sb[:, 0, :], x_d[:, 0, :])
                w13_first = []
                wdr = []
                for wd, wn in ((w1_d, "w1"), (w3_d, "w3")):
                    w_sb = pool.tile([128, DPO, 128], BF16, tag="w13",
                                     bufs=6, name=f"{wn}_{jname}_0")
                    w13_first.append(w_sb)
                    wdr.append(wd[0].rearrange("p (a b) -> p a b", a=DPO))
                for w_sb, wsrc in zip(w13_first, wdr):
                    nc.sync.dma_start(w_sb[:, 0:4, :], wsrc[:, 0:4, :])
                for w_sb, wsrc in zip(w13_first, wdr):
                    nc.sync.dma_start(w_sb[:, 4:, :], wsrc[:, 4:, :])
                for dsl in range(1, DPO):
                    nc.sync.dma_start(x_sb[:, dsl, :], x_d[:, dsl, :])
                if scaled:
                    nc.sync.dma_start(cw_sb[:], cwb[:])
                H = pool.tile([128, IPO, CJ], BF16, tag=f"H_{jname}",
                              bufs=1, name=f"H_{jname}")

                # ---- phase A: H = silu(x@w1T) * (x@w3T) [* cw] ----
                for i in range(IPO):
                    if i == 0:
                        w1_sb, w3_sb = w13_first
                    else:
                        w1_sb = pool.tile([128, DPO, 128], BF16, tag="w13",
                                          bufs=6, name=f"w1_{jname}_{i}")
                        nc.sync.dma_start(
                            w1_sb[:],
                            w1_d[i].rearrange("p (a b) -> p a b", a=DPO))
                        w3_sb = pool.tile([128, DPO, 128], BF16, tag="w13",
                                          bufs=6, name=f"w3_{jname}_{i}")
                        nc.sync.dma_start(
                            w3_sb[:],
                            w3_d[i].rearrange("p (a b) -> p a b", a=DPO))
                    p1s = []
                    p3s = []
                    for bi, (off, w) in enumerate(cbs):
                        p1s.append(pp.tile([128, w], F32, tag="pa", bufs=6,
                                           name=f"p1_{jname}_{i}_{bi}"))
                        p3s.append(pp.tile([128, w], F32, tag="pa", bufs=6,
                                           name=f"p3_{jname}_{i}_{bi}"))
                    for d in range(DPO):
                        for bi, (off, w) in enumerate(cbs):
                            nc.tensor.matmul(
                                p1s[bi][:], w1_sb[:, d, :],
                                x_sb[:, d, off:off + w],
                                start=(d == 0), stop=(d == DPO - 1))
                        for bi, (off, w) in enumerate(cbs):
                            nc.tensor.matmul(
                                p3s[bi][:], w3_sb[:, d, :],
                                x_sb[:, d, off:off + w],
                                start=(d == 0), stop=(d == DPO - 1))
                    for bi, (off, w) in enumerate(cbs):
                        s_t = pool.tile([128, w], F32, tag="act1", bufs=6,
                                        name=f"s_{jname}_{i}_{bi}")
                        nc.scalar.activation(s_t[:], p1s[bi][:], Silu)
                        if scaled:
                            t_t = pool.tile([128, w], F32, tag="act2", bufs=6,
                                            name=f"t_{jname}_{i}_{bi}")
                            nc.vector.tensor_mul(t_t[:], p3s[bi][:],
                                                 cw_sb[:, off:off + w])
                            nc.vector.tensor_mul(H[:, i, off:off + w],
                                                 s_t[:], t_t[:])
                        else:
                            nc.vector.tensor_mul(H[:, i, off:off + w],
                                                 s_t[:], p3s[bi][:])

                # ---- phase B: out = H @ w2T ----
                for do in range(DPO):
                    w2_sb = pool.tile([128, IPO, 128], BF16, tag="w2",
                                      bufs=5, name=f"w2_{jname}_{do}")
                    nc.sync.dma_start(
                        w2_sb[:], w2_d[do].rearrange("p (a b) -> p a b", a=IPO))
                    pys = []
                    for bi, (off, w) in enumerate(cbs):
                        pys.append(pp.tile([128, w], F32, tag="pb", bufs=2,
                                           name=f"py_{jname}_{do}_{bi}"))
                    for i in range(IPO):
                        for bi, (off, w) in enumerate(cbs):
                            nc.tensor.matmul(
                                pys[bi][:], w2_sb[:, i, :],
                                H[:, i, off:off + w],
                                start=(i == 0), stop=(i == IPO - 1))
                    y_t = pool.tile([128, CJ], BF16, tag="yo", bufs=4,
                                    name=f"y_{jname}_{do}")
                    for bi, (off, w) in enumerate(cbs):
                        nc.vector.tensor_copy(y_t[:, off:off + w], pys[bi][:])
                    nc.sync.dma_start(out_d[:, do, :], y_t[:])

    nc.finalize()
    return nc


def _get_kernel(C, TS):
    key = (C, TS)
    if key not in _BUILD_CACHE:
        _BUILD_CACHE[key] = _build(C, TS)
    return _BUILD_CACHE[key]


def _pm(a, po):
    """[N, po*128] -> partition-major [128, po, N] contiguous."""
    n = a.shape[0]
    return np.ascontiguousarray(
        a.T.reshape(po, 128, n).transpose(1, 0, 2))


def kernel(x, gate_w, gate_b, w1, w2, w3, sw1, sw2, sw3):
    bf16 = ml_dtypes.bfloat16
    x = np.asarray(x)
    gate_w = np.asarray(gate_w, dtype=np.float32)
    gate_b = np.asarray(gate_b, dtype=np.float32)
    w1 = np.asarray(w1)
    w2 = np.asarray(w2)
    w3 = np.asarray(w3)
    sw1 = np.asarray(sw1)
    sw2 = np.asarray(sw2)
    sw3 = np.asarray(sw3)

    B, S, Dx = x.shape
    assert Dx == D
    T = B * S
    TS = T // N_CORES
    xt = x.reshape(T, D)

    # ---- gate (fp32, mirrors reference: sqrt(softplus), top-2 on biased) ----
    xf = xt.astype(np.float32)
    logits = xf @ gate_w.T
    scores = np.sqrt(np.log1p(np.exp(-np.abs(logits)))
                     + np.maximum(logits, 0.0))
    biased = scores + gate_b
    idx = np.argsort(-biased, axis=1, kind="stable")[:, :TOPK]
    cw = np.zeros((T, E), dtype=np.float32)
    np.put_along_axis(cw, idx, np.take_along_axis(scores, idx, axis=1), axis=1)

    sel = np.zeros((T, E), dtype=bool)
    np.put_along_axis(sel, idx, True, axis=1)
    tok_lists = [np.nonzero(sel[:, e])[0] for e in range(E)]
    counts = np.array([len(t) for t in tok_lists])
    C = max(256, int(-(-counts.max() // 128) * 128))

    nc = _get_kernel(C, TS)

    # ---- per-core input prep ----
    # weight transforms: lhsT layouts, block-major so DMAs are contiguous
    def wA_layout(wm):  # [I, D] -> [IPO, 128, D]; [ib,pi,po*128+ic]
        return np.ascontiguousarray(
            wm.T.reshape(DPO, 128, IPO, 128).transpose(2, 1, 0, 3)
        ).reshape(IPO, 128, D)

    def wB_layout(wm):  # [D, I] -> [DPO, 128, I]; [db,pi,po*128+dc]
        return np.ascontiguousarray(
            wm.T.reshape(IPO, 128, DPO, 128).transpose(2, 1, 0, 3)
        ).reshape(DPO, 128, I)

    sw1t = wA_layout(sw1)
    sw3t = wA_layout(sw3)
    sw2t = wB_layout(sw2)

    in_maps = []
    for e in range(E):
        toks = tok_lists[e]
        cnt = len(toks)
        xg = np.zeros((C, D), dtype=bf16)
        xg[:cnt] = xt[toks]
        cwe = np.zeros((C,), dtype=np.float32)
        cwe[:cnt] = cw[toks, e]
        xs_slice = xt[e * TS:(e + 1) * TS]
        in_maps.append({
            "xr": _pm(xg, DPO),
            "xs": _pm(xs_slice, DPO),
            "cwb": np.ascontiguousarray(
                np.broadcast_to(cwe[None, :], (128, C))),
            "w1t": wA_layout(w1[e]),
            "w3t": wA_layout(w3[e]),
            "w2t": wB_layout(w2[e]),
            "sw1t": sw1t,
            "sw3t": sw3t,
            "sw2t": sw2t,
        })

    res = bass_utils.run_bass_kernel_spmd(
        nc, in_maps, core_ids=list(range(N_CORES)))
    global LAST_RESULT
    LAST_RESULT = res

    # ---- unshard + combine (bf16, reference addition order) ----
    y = np.zeros((T, D), dtype=bf16)
    for e in range(E):
        toks = tok_lists[e]
        cnt = len(toks)
        ye = res.results[e]["ye"]                       # [128, DPO, C]
        ye_tok = ye.transpose(2, 1, 0).reshape(C, D)    # [c, d]
        y[toks] = y[toks] + ye_tok[:cnt]
    z = np.concatenate(
        [res.results[e]["zs"].transpose(2, 1, 0).reshape(TS, D)
         for e in range(E)], axis=0)
    out = (y + z).reshape(B, S, D)
    return out.astype(x.dtype)


# revision 18
# speedup vs baseline: 1.1080x; 1.1080x over previous
"""MoE (8 routed experts, top-2, + shared expert) on 8 TRN2 NeuronCores.

Strategy: expert-parallel. Host computes the gate (fp32 numpy, exactly
mirroring the reference), gathers each expert's tokens, and core e runs
expert e's SwiGLU (h = silu(x@w1T) * (x@w3T) * cw; y = h_bf16 @ w2T)
over its gathered tokens, plus a 1/8 token-slice of the shared expert.
Host scatters expert outputs back and combines in bf16 expert order.

All tensors fed to the device are pre-arranged on host into
partition-major layouts so every DMA is contiguous per partition:
  activations/weights for matmul lhsT/rhs always have the contraction
  dim chunked as [pi=128, po, free].
"""

import numpy as np
import ml_dtypes

import concourse.mybir as mybir
from concourse import bacc
from concourse.tile import TileContext
from concourse import bass_utils

BF16 = mybir.dt.bfloat16
F32 = mybir.dt.float32

D = 2048          # model dim
I = 1408          # expert inter dim
E = 8             # routed experts
TOPK = 2
N_CORES = 8
DPO = D // 128    # 16 chunks of the model dim
IPO = I // 128    # 11 chunks of the inter dim

_BUILD_CACHE = {}


def _c_blocks(C):
    """Split C columns into equal-ish blocks <= 512, multiples of 128."""
    nb = -(-C // 512)
    per = -(-C // (nb * 128)) * 128
    blocks = []
    off = 0
    while off < C:
        w = min(per, C - off)
        blocks.append((off, w))
        off += w
    return blocks


def _build(C, TS):
    """Build the per-core Bass kernel for routed capacity C and shared
    token-slice TS. Same NEFF runs SPMD on all 8 cores."""
    nc = bacc.Bacc("TRN2", debug=False, enable_asserts=False,
                   num_devices=N_CORES, enable_partition_id=False)

    def din(name, shape, dt=BF16):
        return nc.dram_tensor(name, shape, dt, kind="ExternalInput").ap()

    def dout(name, shape, dt=BF16):
        return nc.dram_tensor(name, shape, dt, kind="ExternalOutput").ap()

    xr = din("xr", [128, DPO, C])            # routed tokens, [d_pi, d_po, c]
    xs = din("xs", [128, DPO, TS])           # shared-expert token slice
    cwb = din("cwb", [128, C], F32)          # combine weights, replicated
    w1t = din("w1t", [IPO, 128, D])          # [i_blk][d_pi][d_po*128+i_c]
    w3t = din("w3t", [IPO, 128, D])
    w2t = din("w2t", [DPO, 128, I])          # [d_blk][i_pi][i_po*128+d_c]
    sw1t = din("sw1t", [IPO, 128, D])
    sw3t = din("sw3t", [IPO, 128, D])
    sw2t = din("sw2t", [DPO, 128, I])
    ye = dout("ye", [128, DPO, C])           # [d_pi, d_po, c]
    zs = dout("zs", [128, DPO, TS])

    Silu = mybir.ActivationFunctionType.Silu

    with TileContext(nc) as tc:
        with tc.tile_pool(name="main", bufs=1) as pool, \
             tc.tile_pool(name="psum", bufs=1, space="PSUM") as pp:
            cw_sb = pool.tile([128, C], F32, tag="cwb", bufs=1, name="cw_sb")

            # routed job first: the second job's startup stream then
            # overlaps the first job's ~113us of phase-B PE work, and the
            # small xs stream doesn't starve routed phase-B weight loads
            jobs = [
                ("r", C, xr, w1t, w3t, w2t, ye, True),
                ("s", TS, xs, sw1t, sw3t, sw2t, zs, False),
            ]
            for jname, CJ, x_d, w1_d, w3_d, w2_d, out_d, scaled in jobs:
                cbs = _c_blocks(CJ)
                x_sb = pool.tile([128, DPO, CJ], BF16, tag=f"x_{jname}",
                                 bufs=1, name=f"x_{jname}")
                # startup: land just enough bytes for the first matmuls
                # (x slice 0 + the first weight chunks) before streaming
                # the rest, so the PE starts ~10us in instead of ~25us
                nc.sync.dma_start(x_sb[:, 0, :], x_d[:, 0, :])
                w13_first = []
                wdr = []
                for wd, wn in ((w1_d, "w1"), (w3_d, "w3")):
                    w_sb = pool.tile([128, DPO, 128], BF16, tag="w13",
                                     bufs=6, name=f"{wn}_{jname}_0")
                    w13_first.append(w_sb)
                    wdr.append(wd[0].rearrange("p (a b) -> p a b", a=DPO))
                for w_sb, wsrc in zip(w13_first, wdr):
                    nc.sync.dma_start(w_sb[:, 0:4, :], wsrc[:, 0:4, :])
                for w_sb, wsrc in zip(w13_first, wdr):
                    nc.sync.dma_start(w_sb[:, 4:, :], wsrc[:, 4:, :])
                for dsl in range(1, DPO):
                    nc.sync.dma_start(x_sb[:, dsl, :], x_d[:, dsl, :])
                if scaled:
                    nc.sync.dma_start(cw_sb[:], cwb[:])
                H = pool.tile([128, IPO, CJ], BF16, tag=f"H_{jname}",
                              bufs=1, name=f"H_{jname}")

                # ---- phase A: H = silu(x@w1T) * (x@w3T) [* cw] ----
                for i in range(IPO):
                    if i == 0:
                        w1_sb, w3_sb = w13_first
                    else:
                        w1_sb = pool.tile([128, DPO, 128], BF16, tag="w13",
                                          bufs=6, name=f"w1_{jname}_{i}")
                        nc.sync.dma_start(
                            w1_sb[:],
                            w1_d[i].rearrange("p (a b) -> p a b", a=DPO))
                        w3_sb = pool.tile([128, DPO, 128], BF16, tag="w13",
                                          bufs=6, name=f"w3_{jname}_{i}")
                        nc.sync.dma_start(
                            w3_sb[:],
                            w3_d[i].rearrange("p (a b) -> p a b", a=DPO))
                    p1s = []
                    p3s = []
                    for bi, (off, w) in enumerate(cbs):
                        p1s.append(pp.tile([128, w], F32, tag="pa", bufs=6,
                                           name=f"p1_{jname}_{i}_{bi}"))
                        p3s.append(pp.tile([128, w], F32, tag="pa", bufs=6,
                                           name=f"p3_{jname}_{i}_{bi}"))
                    for d in range(DPO):
                        for bi, (off, w) in enumerate(cbs):
                            nc.tensor.matmul(
                                p1s[bi][:], w1_sb[:, d, :],
                                x_sb[:, d, off:off + w],
                                start=(d == 0), stop=(d == DPO - 1))
                        for bi, (off, w) in enumerate(cbs):
                            nc.tensor.matmul(
                                p3s[bi][:], w3_sb[:, d, :],
                                x_sb[:, d, off:off + w],
                                start=(d == 0), stop=(d == DPO - 1))
                    for bi, (off, w) in enumerate(cbs):
                        s_t = pool.tile([128, w], F32, tag="act1", bufs=6,
                                        name=f"s_{jname}_{i}_{bi}")
                        nc.scalar.activation(s_t[:], p1s[bi][:], Silu)
                        if scaled:
                            t_t = pool.tile([128, w], F32, tag="act2", bufs=6,
                                            name=f"t_{jname}_{i}_{bi}")
                            nc.vector.tensor_mul(t_t[:], p3s[bi][:],
                                                 cw_sb[:, off:off + w])
                            nc.vector.tensor_mul(H[:, i, off:off + w],
                                                 s_t[:], t_t[:])
                        else:
                            nc.vector.tensor_mul(H[:, i, off:off + w],
                                                 s_t[:], p3s[bi][:])

                # ---- phase B: out = H @ w2T ----
                for do in range(DPO):
                    w2_sb = pool.tile([128, IPO, 128], BF16, tag="w2",
                                      bufs=5, name=f"w2_{jname}_{do}")
                    nc.sync.dma_start(
                        w2_sb[:], w2_d[do].rearrange("p (a b) -> p a b", a=IPO))
                    pys = []
                    for bi, (off, w) in enumerate(cbs):
                        pys.append(pp.tile([128, w], F32, tag="pb", bufs=2,
                                           name=f"py_{jname}_{do}_{bi}"))
                    for i in range(IPO):
                        for bi, (off, w) in enumerate(cbs):
                            nc.tensor.matmul(
                                pys[bi][:], w2_sb[:, i, :],
                                H[:, i, off:off + w],
                                start=(i == 0), stop=(i == IPO - 1))
                    y_t = pool.tile([128, CJ], BF16, tag="yo", bufs=4,
                                    name=f"y_{jname}_{do}")
                    for bi, (off, w) in enumerate(cbs):
                        nc.vector.tensor_copy(y_t[:, off:off + w], pys[bi][:])
                    nc.sync.dma_start(out_d[:, do, :], y_t[:])

    nc.finalize()
    return nc


def _get_kernel(C, TS):
    key = (C, TS)
    if key not in _BUILD_CACHE:
        _BUILD_CACHE[key] = _build(C, TS)
    return _BUILD_CACHE[key]


def _pm(a, po):
    """[N, po*128] -> partition-major [128, po, N] contiguous."""
    n = a.shape[0]
    return np.ascontiguousarray(
        a.T.reshape(po, 128, n).transpose(1, 0, 2))


def kernel(x, gate_w, gate_b, w1, w2, w3, sw1, sw2, sw3):
    bf16 = ml_dtypes.bfloat16
    x = np.asarray(x)
    gate_w = np.asarray(gate_w, dtype=np.float32)
    gate_b = np.asarray(gate_b, dtype=np.float32)
    w1 = np.asarray(w1)
    w2 = np.asarray(w2)
    w3 = np.asarray(w3)
    sw1 = np.asarray(sw1)
    sw2 = np.asarray(sw2)
    sw3 = np.asarray(sw3)

    B, S, Dx = x.shape
    assert Dx == D
    T = B * S
    TS = T // N_CORES
    xt = x.reshape(T, D)

    # ---- gate (fp32, mirrors reference: sqrt(softplus), top-2 on biased) ----
    xf = xt.astype(np.float32)
    logits = xf @ gate_w.T
    scores = np.sqrt(np.log1p(np.exp(-np.abs(logits)))
                     + np.maximum(logits, 0.0))
    biased = scores + gate_b
    idx = np.argsort(-biased, axis=1, kind="stable")[:, :TOPK]
    cw = np.zeros((T, E), dtype=np.float32)
    np.put_along_axis(cw, idx, np.take_along_axis(scores, idx, axis=1), axis=1)

    sel = np.zeros((T, E), dtype=bool)
    np.put_along_axis(sel, idx, True, axis=1)
    tok_lists = [np.nonzero(sel[:, e])[0] for e in range(E)]
    counts = np.array([len(t) for t in tok_lists])
    C = max(256, int(-(-counts.max() // 128) * 128))

    nc = _get_kernel(C, TS)

    # ---- per-core input prep ----
    # weight transforms: lhsT layouts, block-major so DMAs are contiguous
    def wA_layout(wm):  # [I, D] -> [IPO, 128, D]; [ib,pi,po*128+ic]
        return np.ascontiguousarray(
            wm.T.reshape(DPO, 128, IPO, 128).transpose(2, 1, 0, 3)
        ).reshape(IPO, 128, D)

    def wB_layout(wm):  # [D, I] -> [DPO, 128, I]; [db,pi,po*128+dc]
        return np.ascontiguousarray(
            wm.T.reshape(IPO, 128, DPO, 128).transpose(2, 1, 0, 3)
        ).reshape(DPO, 128, I)

    sw1t = wA_layout(sw1)
    sw3t = wA_layout(sw3)
    sw2t = wB_layout(sw2)

    in_maps = []
    for e in range(E):
        toks = tok_lists[e]
        cnt = len(toks)
        xg = np.zeros((C, D), dtype=bf16)
        xg[:cnt] = xt[toks]
        cwe = np.zeros((C,), dtype=np.float32)
        cwe[:cnt] = cw[toks, e]
        xs_slice = xt[e * TS:(e + 1) * TS]
        in_maps.append({
            "xr": _pm(xg, DPO),
            "xs": _pm(xs_slice, DPO),
            "cwb": np.ascontiguousarray(
                np.broadcast_to(cwe[None, :], (128, C))),
            "w1t": wA_layout(w1[e]),
            "w3t": wA_layout(w3[e]),
            "w2t": wB_layout(w2[e]),
            "sw1t": sw1t,
            "sw3t": sw3t,
            "sw2t": sw2t,
        })

    res = bass_utils.run_bass_kernel_spmd(
        nc, in_maps, core_ids=list(range(N_CORES)))
    global LAST_RESULT
    LAST_RESULT = res

    # ---- unshard + combine (bf16, reference addition order) ----
    y = np.zeros((T, D), dtype=bf16)
    for e in range(E):
        toks = tok_lists[e]
        cnt = len(toks)
        ye = res.results[e]["ye"]                       # [128, DPO, C]
        ye_tok = ye.transpose(2, 1, 0).reshape(C, D)    # [c, d]
        y[toks] = y[toks] + ye_tok[:cnt]
    z = np.concatenate(
        [res.results[e]["zs"].transpose(2, 1, 0).reshape(TS, D)
         for e in range(E)], axis=0)
    out = (y + z).reshape(B, S, D)
    return out.astype(x.dtype)


# revision 19
# speedup vs baseline: 1.1081x; 1.0001x over previous
"""MoE (8 routed experts, top-2, + shared expert) on 8 TRN2 NeuronCores.

Strategy: expert-parallel. Host computes the gate (fp32 numpy, exactly
mirroring the reference), gathers each expert's tokens, and core e runs
expert e's SwiGLU (h = silu(x@w1T) * (x@w3T) * cw; y = h_bf16 @ w2T)
over its gathered tokens, plus a 1/8 token-slice of the shared expert.
Host scatters expert outputs back and combines in bf16 expert order.

All tensors fed to the device are pre-arranged on host into
partition-major layouts so every DMA is contiguous per partition:
  activations/weights for matmul lhsT/rhs always have the contraction
  dim chunked as [pi=128, po, free].
"""

import numpy as np
import ml_dtypes

import concourse.mybir as mybir
from concourse import bacc
from concourse.tile import TileContext
from concourse import bass_utils

BF16 = mybir.dt.bfloat16
F32 = mybir.dt.float32

D = 2048          # model dim
I = 1408          # expert inter dim
E = 8             # routed experts
TOPK = 2
N_CORES = 8
DPO = D // 128    # 16 chunks of the model dim
IPO = I // 128    # 11 chunks of the inter dim

_BUILD_CACHE = {}


def _c_blocks(C):
    """Split C columns into equal-ish blocks <= 512, multiples of 128."""
    nb = -(-C // 512)
    per = -(-C // (nb * 128)) * 128
    blocks = []
    off = 0
    while off < C:
        w = min(per, C - off)
        blocks.append((off, w))
        off += w
    return blocks


def _build(C, TS):
    """Build the per-core Bass kernel for routed capacity C and shared
    token-slice TS. Same NEFF runs SPMD on all 8 cores."""
    nc = bacc.Bacc("TRN2", debug=False, enable_asserts=False,
                   num_devices=N_CORES, enable_partition_id=False)

    def din(name, shape, dt=BF16):
        return nc.dram_tensor(name, shape, dt, kind="ExternalInput").ap()

    def dout(name, shape, dt=BF16):
        return nc.dram_tensor(name, shape, dt, kind="ExternalOutput").ap()

    xr = din("xr", [128, DPO, C])            # routed tokens, [d_pi, d_po, c]
    xs = din("xs", [128, DPO, TS])           # shared-expert token slice
    cwb = din("cwb", [128, C], F32)          # combine weights, replicated
    w1t = din("w1t", [IPO, 128, D])          # [i_blk][d_pi][d_po*128+i_c]
    w3t = din("w3t", [IPO, 128, D])
    w2t = din("w2t", [DPO, 128, I])          # [d_blk][i_pi][i_po*128+d_c]
    sw1t = din("sw1t", [IPO, 128, D])
    sw3t = din("sw3t", [IPO, 128, D])
    sw2t = din("sw2t", [DPO, 128, I])
    ye = dout("ye", [128, DPO, C])           # [d_pi, d_po, c]
    zs = dout("zs", [128, DPO, TS])

    Silu = mybir.ActivationFunctionType.Silu

    with TileContext(nc) as tc:
        with tc.tile_pool(name="main", bufs=1) as pool, \
             tc.tile_pool(name="psum", bufs=1, space="PSUM") as pp:
            cw_sb = pool.tile([128, C], F32, tag="cwb", bufs=1, name="cw_sb")

            # routed job first: the second job's startup stream then
            # overlaps the first job's ~113us of phase-B PE work, and the
            # small xs stream doesn't starve routed phase-B weight loads
            jobs = [
                ("r", C, xr, w1t, w3t, w2t, ye, True),
                ("s", TS, xs, sw1t, sw3t, sw2t, zs, False),
            ]
            for jname, CJ, x_d, w1_d, w3_d, w2_d, out_d, scaled in jobs:
                cbs = _c_blocks(CJ)
                x_sb = pool.tile([128, DPO, CJ], BF16, tag=f"x_{jname}",
                                 bufs=1, name=f"x_{jname}")
                # startup: land just enough bytes for the first matmuls
                # (x slice 0 + the first weight chunks) before streaming
                # the rest, so the PE starts ~10us in instead of ~25us
                nc.sync.dma_start(x_sb[:, 0, :], x_d[:, 0, :])
                w13_first = []
                wdr = []
                for wd, wn in ((w1_d, "w1"), (w3_d, "w3")):
                    w_sb = pool.tile([128, DPO, 128], BF16, tag="w13",
                                     bufs=8, name=f"{wn}_{jname}_0")
                    w13_first.append(w_sb)
                    wdr.append(wd[0].rearrange("p (a b) -> p a b", a=DPO))
                for w_sb, wsrc in zip(w13_first, wdr):
                    nc.sync.dma_start(w_sb[:, 0:4, :], wsrc[:, 0:4, :])
                for w_sb, wsrc in zip(w13_first, wdr):
                    nc.sync.dma_start(w_sb[:, 4:, :], wsrc[:, 4:, :])
                for dsl in range(1, DPO):
                    nc.sync.dma_start(x_sb[:, dsl, :], x_d[:, dsl, :])
                if scaled:
                    nc.sync.dma_start(cw_sb[:], cwb[:])
                H = pool.tile([128, IPO, CJ], BF16, tag=f"H_{jname}",
                              bufs=1, name=f"H_{jname}")

                # ---- phase A: H = silu(x@w1T) * (x@w3T) [* cw] ----
                for i in range(IPO):
                    if i == 0:
                        w1_sb, w3_sb = w13_first
                    else:
                        w1_sb = pool.tile([128, DPO, 128], BF16, tag="w13",
                                          bufs=8, name=f"w1_{jname}_{i}")
                        nc.sync.dma_start(
                            w1_sb[:],
                            w1_d[i].rearrange("p (a b) -> p a b", a=DPO))
                        w3_sb = pool.tile([128, DPO, 128], BF16, tag="w13",
                                          bufs=8, name=f"w3_{jname}_{i}")
                        nc.sync.dma_start(
                            w3_sb[:],
                            w3_d[i].rearrange("p (a b) -> p a b", a=DPO))
                    p1s = []
                    p3s = []
                    for bi, (off, w) in enumerate(cbs):
                        p1s.append(pp.tile([128, w], F32, tag="pa", bufs=6,
                                           name=f"p1_{jname}_{i}_{bi}"))
                        p3s.append(pp.tile([128, w], F32, tag="pa", bufs=6,
                                           name=f"p3_{jname}_{i}_{bi}"))
                    for d in range(DPO):
                        for bi, (off, w) in enumerate(cbs):
                            nc.tensor.matmul(
                                p1s[bi][:], w1_sb[:, d, :],
                                x_sb[:, d, off:off + w],
                                start=(d == 0), stop=(d == DPO - 1))
                        for bi, (off, w) in enumerate(cbs):
                            nc.tensor.matmul(
                                p3s[bi][:], w3_sb[:, d, :],
                                x_sb[:, d, off:off + w],
                                start=(d == 0), stop=(d == DPO - 1))
                    for bi, (off, w) in enumerate(cbs):
                        s_t = pool.tile([128, w], F32, tag="act1", bufs=6,
                                        name=f"s_{jname}_{i}_{bi}")
                        nc.scalar.activation(s_t[:], p1s[bi][:], Silu)
                        if scaled:
                            t_t = pool.tile([128, w], F32, tag="act2", bufs=6,
                                            name=f"t_{jname}_{i}_{bi}")
                            nc.vector.tensor_mul(t_t[:], p3s[bi][:],
                                                 cw_sb[:, off:off + w])
                            nc.vector.tensor_mul(H[:, i, off:off + w],
                                                 s_t[:], t_t[:])
                        else:
                            nc.vector.tensor_mul(H[:, i, off:off + w],
                                                 s_t[:], p3s[bi][:])

                # ---- phase B: out = H @ w2T ----
                for do in range(DPO):
                    w2_sb = pool.tile([128, IPO, 128], BF16, tag="w2",
                                      bufs=6, name=f"w2_{jname}_{do}")
                    nc.sync.dma_start(
                        w2_sb[:], w2_d[do].rearrange("p (a b) -> p a b", a=IPO))
                    pys = []
                    for bi, (off, w) in enumerate(cbs):
                        pys.append(pp.tile([128, w], F32, tag="pb", bufs=2,
                                           name=f"py_{jname}_{do}_{bi}"))
                    for i in range(IPO):
                        for bi, (off, w) in enumerate(cbs):
                            nc.tensor.matmul(
                                pys[bi][:], w2_sb[:, i, :],
                                H[:, i, off:off + w],
                                start=(i == 0), stop=(i == IPO - 1))
                    y_t = pool.tile([128, CJ], BF16, tag="yo", bufs=6,
                                    name=f"y_{jname}_{do}")
                    for bi, (off, w) in enumerate(cbs):
                        nc.vector.tensor_copy(y_t[:, off:off + w], pys[bi][:])
                    nc.sync.dma_start(out_d[:, do, :], y_t[:])

    nc.finalize()
    return nc


def _get_kernel(C, TS):
    key = (C, TS)
    if key not in _BUILD_CACHE:
        _BUILD_CACHE[key] = _build(C, TS)
    return _BUILD_CACHE[key]


def _pm(a, po):
    """[N, po*128] -> partition-major [128, po, N] contiguous."""
    n = a.shape[0]
    return np.ascontiguousarray(
        a.T.reshape(po, 128, n).transpose(1, 0, 2))


def kernel(x, gate_w, gate_b, w1, w2, w3, sw1, sw2, sw3):
    bf16 = ml_dtypes.bfloat16
    x = np.asarray(x)
    gate_w = np.asarray(gate_w, dtype=np.float32)
    gate_b = np.asarray(gate_b, dtype=np.float32)
    w1 = np.asarray(w1)
    w2 = np.asarray(w2)
    w3 = np.asarray(w3)
    sw1 = np.asarray(sw1)
    sw2 = np.asarray(sw2)
    sw3 = np.asarray(sw3)

    B, S, Dx = x.shape
    assert Dx == D
    T = B * S
    TS = T // N_CORES
    xt = x.reshape(T, D)

    # ---- gate (fp32, mirrors reference: sqrt(softplus), top-2 on biased) ----
    xf = xt.astype(np.float32)
    logits = xf @ gate_w.T
    scores = np.sqrt(np.log1p(np.exp(-np.abs(logits)))
                     + np.maximum(logits, 0.0))
    biased = scores + gate_b
    idx = np.argsort(-biased, axis=1, kind="stable")[:, :TOPK]
    cw = np.zeros((T, E), dtype=np.float32)
    np.put_along_axis(cw, idx, np.take_along_axis(scores, idx, axis=1), axis=1)

    sel = np.zeros((T, E), dtype=bool)
    np.put_along_axis(sel, idx, True, axis=1)
    tok_lists = [np.nonzero(sel[:, e])[0] for e in range(E)]
    counts = np.array([len(t) for t in tok_lists])
    C = max(256, int(-(-counts.max() // 128) * 128))

    nc = _get_kernel(C, TS)

    # ---- per-core input prep ----
    # weight transforms: lhsT layouts, block-major so DMAs are contiguous
    def wA_layout(wm):  # [I, D] -> [IPO, 128, D]; [ib,pi,po*128+ic]
        return np.ascontiguousarray(
            wm.T.reshape(DPO, 128, IPO, 128).transpose(2, 1, 0, 3)
        ).reshape(IPO, 128, D)

    def wB_layout(wm):  # [D, I] -> [DPO, 128, I]; [db,pi,po*128+dc]
        return np.ascontiguousarray(
            wm.T.reshape(IPO, 128, DPO, 128).transpose(2, 1, 0, 3)
        ).reshape(DPO, 128, I)

    sw1t = wA_layout(sw1)
    sw3t = wA_layout(sw3)
    sw2t = wB_layout(sw2)

    in_maps = []
    for e in range(E):
        toks = tok_lists[e]
        cnt = len(toks)
        xg = np.zeros((C, D), dtype=bf16)
        xg[:cnt] = xt[toks]
        cwe = np.zeros((C,), dtype=np.float32)
        cwe[:cnt] = cw[toks, e]
        xs_slice = xt[e * TS:(e + 1) * TS]
        in_maps.append({
            "xr": _pm(xg, DPO),
            "xs": _pm(xs_slice, DPO),
            "cwb": np.ascontiguousarray(
                np.broadcast_to(cwe[None, :], (128, C))),
            "w1t": wA_layout(w1[e]),
            "w3t": wA_layout(w3[e]),
            "w2t": wB_layout(w2[e]),
            "sw1t": sw1t,
            "sw3t": sw3t,
            "sw2t": sw2t,
        })

    res = bass_utils.run_bass_kernel_spmd(
        nc, in_maps, core_ids=list(range(N_CORES)))
    global LAST_RESULT
    LAST_RESULT = res

    # ---- unshard + combine (bf16, reference addition order) ----
    y = np.zeros((T, D), dtype=bf16)
    for e in range(E):
        toks = tok_lists[e]
        cnt = len(toks)
        ye = res.results[e]["ye"]                       # [128, DPO, C]
        ye_tok = ye.transpose(2, 1, 0).reshape(C, D)    # [c, d]
        y[toks] = y[toks] + ye_tok[:cnt]
    z = np.concatenate(
        [res.results[e]["zs"].transpose(2, 1, 0).reshape(TS, D)
         for e in range(E)], axis=0)
    out = (y + z).reshape(B, S, D)
    return out.astype(x.dtype)
